# revision 65
# baseline (speedup 1.0000x reference)
"""CisAttentionLayer Trainium2 kernel — 8-core SPMD via bass/Tile.

Sharding: core = (batch b, head-half hh). Each core computes 4 heads x all
1024 genes for one batch. The final output-projection partials from the two
cores of a batch are summed on the host during unshard, where the (cheap,
output-sized) layernorm also runs in f64.

Key optimizations over the dense baseline:
  - SNP compaction: the padding mask kills ~50% of the 4096 SNPs for every
    gene/head of a batch, so the host gathers only valid SNPs (padded to
    NCH*128). All attention work (projections, scores, exp, attn@V) halves.
  - Head-parallel sharding halves the K/V projections instead of
    duplicating them per gene-half.
  - attn@V computed in [genes, d_k] layout (genes on PSUM partitions,
    N=64 free) — half the PE cycles of the [d_k, genes] layout.
  - Row sums via an appended ones-column on V (65-wide attn@V matmuls).
  - Scores processed as two 512-gene halves per (pair,c,hl) unit: each
    half is one PSUM bank, giving a 6-deep ss ring (vs 3 full-width) so
    three element-wise engines stay fed; attn@V consumes per-half via
    subtile deps.
  - exp split across engines by a static balanced schedule (GPSIMD
    cannot read PSUM, so only ACT/DVE touch scores): ~42/68 units use
    ACT native exp with the cis multiply on DVE (2x-mode fp16 TT,
    per-half) or GpSimd (SBUF-only TT); ~26 units use a fused DVE
    bit-trick exp ((score*A*SCALE + A*SCALE*dqK + B) as int16, bitcast
    to fp16 ~= exp((score+dqK)*SCALE), cis multiply riding in the same
    scalar_tensor_tensor). One fp16 {0,1} mask tile serves all paths
    (and kills the compaction padding).
  - attn@V accumulators for 8 gene-tiles share PSUM banks; the bank is
    pending-zeroed with a free N=1 start=True matmul (HW start=True
    zeroes the whole 2KB bank!) and all accumulating matmuls use
    start=False.
  - Phase A (projections) and the first head's attention units are
    emitted interleaved, with kv/cis DMAs staged in chunk-aligned pieces
    so PE and the element-wise engines start as early as possible.
  - Softmax normalize: one batched reciprocal + one gt-broadcast
    (stride-0) tensor_tensor per head on DVE.
  - The dq environment shift is algebraically reduced to a per-SNP bias
    dqK[h,s] = (wq_b+dq)_h . K_h[s], computed on-device from fp16 kv with
    tiny N=4 matmuls (u_h = (wq_b+dq)_h @ wk_h precomputed on host).
  - Output transpose ([g,dk] -> [dk,g]): pair 0 via DMA-XBAR transposes
    on the SP queue mid-kernel (dma_start on scalar/vector stalls that
    engine's sequencer ~667ns); pair 1 (the tail) via PE transposes with
    a host-provided identity, keeping the critical path off the serial
    HWDGE descriptor queue. Output DMA batched 2 gene-tiles/descriptor.
"""
import numpy as np
import concourse.bass as bass
import concourse.tile as tile
from concourse import mybir
from concourse.bass_utils import run_bass_kernel_spmd
from concourse.vector_clock import ScopedClock

B, G, S, D, H, DK = 4, 1024, 4096, 512, 8, 64
N_CORES = 8
HPC = 4                 # heads per core
DHC = HPC * DK          # head dims per core (256)
SCALE = 1.0 / np.sqrt(DK).astype(np.float32)   # 0.125
A_T = 1024.0 / np.log(2.0)                     # fp16 bit-trick exp slope
B_T = 15360.0 - 45.0                           # bit-trick intercept (calibrated)
AS = float(A_T * SCALE)                        # QT pre-scale
KILL_I16 = 31743                               # int16; as fp16 bits = 65504.0
MIN_NCH = 17                                   # 17*128 = 2176 >= max valid SNPs

F32 = mybir.dt.float32
F16 = mybir.dt.float16
I16 = mybir.dt.int16
AF = mybir.ActivationFunctionType
ALU = mybir.AluOpType


# ---------------------------------------------------------------------------
# Tile compat: this container's walrus rejects >1 sync wait per instruction.
# ---------------------------------------------------------------------------
def _split_sync_waits(nc):
    for f in nc.m.functions:
        for bb in f.blocks:
            idx = 0
            while idx < len(bb.instructions):
                inst = bb.instructions[idx]
                si = inst.sync_info
                if si is not None and len(si.on_wait) > 1:
                    waits = list(si.on_wait)
                    for w in waits[:-1]:
                        nop = mybir.InstNoOp(
                            name=nc.get_next_instruction_name(),
                            sync_info=mybir.SyncInfo(on_wait=[w], on_update=[]),
                            bass_nofuse=True,
                            engine=inst.engine,
                        )
                        nc.register_instruction(nop)
                        bb.instructions.insert(idx, nop)
                        idx += 1
                    inst.sync_info = mybir.SyncInfo(
                        on_wait=[waits[-1]], on_update=list(si.on_update)
                    )
                idx += 1


class _SafeTileContext(tile.TileContext):
    def _drain_and_barrier(self, tick_clock, wait_clock):
        drain_inst = self.nc.sync.drain()
        wait_clock.add_sem_waits(
            drain_inst.ins, ScopedClock({None: tick_clock.global_clock})
        )
        si = drain_inst.ins.sync_info
        if si is not None and len(si.on_wait) > 1:
            waits = list(si.on_wait)
            drain_inst.ins.sync_info = mybir.SyncInfo(
                on_wait=[waits[0]], on_update=list(si.on_update)
            )
            for w in waits[1:]:
                extra = self.nc.sync.drain()
                extra.ins.sync_info = mybir.SyncInfo(on_wait=[w], on_update=[])
        self.nc.all_engine_barrier()
        assert self.sems is not None
        popped = self.nc._tile_sem_poison_stack.pop()
        assert popped is self._sem_poison
        self.nc.clear_and_free_semaphores(list(self.sems.allocated().values()))
        self.nc.all_engine_barrier()


# ---------------------------------------------------------------------------
# Engine assignment for the (pair,c,hl) units (each = two 512-gene halves).
#
# GPSIMD cannot read PSUM, so the score tensor is consumed only by ACT
# (exp) or DVE (fused bit-trick STT); GpSimd contributes the SBUF-only
# cis-mask multiply for ACT-path units. Types, spread evenly (Bresenham):
#   'actdve'   ACT exp x2 halves + DVE fp16 2x-mode half-masks
#   'actpool'  ACT exp x2 halves + one GpSimd full-width mask
#   'trickdve' DVE fused bit-trick STT x2 halves
# Counts solve ACT ~= DVE(+flush) ~= Pool busy time in phase B.
# ---------------------------------------------------------------------------
def _assignments(nch):
    n = 2 * nch * 2
    counts = {'actdve': 26, 'actpool': 16, 'trickdve': 26}
    if n != 68:  # rescale for other nch
        tot = sum(counts.values())
        counts = {k: max(1, round(v * n / tot)) for k, v in counts.items()}
        counts['actdve'] += n - sum(counts.values())
    sched = []
    acc = {k: 0.0 for k in counts}
    for _ in range(n):
        # pick the type furthest behind its target rate
        k = max(counts, key=lambda t: counts[t] / n * (len(sched) + 1) - acc[t])
        sched.append(k)
        acc[k] += 1
    return sched


# ---------------------------------------------------------------------------
# Kernel build
# ---------------------------------------------------------------------------
def build_nc(nch=MIN_NCH, debug=False):
    sc = nch * 128          # padded compacted SNP count
    nj = (sc + 511) // 512  # 512-wide K-proj column chunks
    sched = _assignments(nch)
    # drain: DVE also runs the last flush, so finish on ACT/Pool paths
    sched[-3:] = ['actdve', 'actpool', 'actdve']

    nc = bass.Bass()
    kvT_d = nc.dram_tensor("kvT", [D, sc], F16, kind="ExternalInput")
    qT_d = nc.dram_tensor("qT", [D, G], F16, kind="ExternalInput")
    wqT_d = nc.dram_tensor("wqT", [D, DHC], F16, kind="ExternalInput")
    wkT_d = nc.dram_tensor("wkT", [D, DHC], F16, kind="ExternalInput")
    wv_d = nc.dram_tensor("wv", [D, DHC], F16, kind="ExternalInput")
    u2_d = nc.dram_tensor("u2", [D, HPC], F16, kind="ExternalInput")
    woT_d = nc.dram_tensor("woT", [DHC, D], F16, kind="ExternalInput")
    cis_d = nc.dram_tensor("cisk", [sc, G], F16, kind="ExternalInput")
    ident_d = nc.dram_tensor("ident", [128, 128], F16, kind="ExternalInput")
    out_d = nc.dram_tensor("out", [G, D], F16, kind="ExternalOutput")
    if debug:
        dbg = {
            'dKT': nc.dram_tensor("dKT", [2, 128, sc], F16, kind="ExternalOutput"),
            'dQT': nc.dram_tensor("dQT", [2, 128, G], F16, kind="ExternalOutput"),
            'dVA': nc.dram_tensor("dVA", [128, nch * HPC * 65], F16, kind="ExternalOutput"),
            'dqa': nc.dram_tensor("dqa", [128, nch * HPC], F32, kind="ExternalOutput"),
            'dqt': nc.dram_tensor("dqt", [128, nch * HPC], F32, kind="ExternalOutput"),
            'dat0': nc.dram_tensor("dat0", [128, G], I16, kind="ExternalOutput"),
            'dat1': nc.dram_tensor("dat1", [128, G], I16, kind="ExternalOutput"),
            'dao': nc.dram_tensor("dao", [128, 8 * 128], F16, kind="ExternalOutput"),
            'daoT': nc.dram_tensor("daoT", [128, 16 * 128], F16, kind="ExternalOutput"),
        }

    with _SafeTileContext(nc) as tc:
        with tc.tile_pool(name="res", bufs=1) as res, \
             tc.tile_pool(name="pb", bufs=11) as pb, \
             tc.tile_pool(name="ao", bufs=2) as aop, \
             tc.tile_pool(name="big", bufs=6, space="PSUM") as bigp, \
             tc.tile_pool(name="acc", bufs=1, space="PSUM") as accp:

            # ---- resident SBUF tensors ----
            kvT4 = res.tile([128, 4, sc], F16, tag="kv", name="kv")
            wkT4 = res.tile([128, 4, DHC], F16, tag="wk", name="wk")
            wvt4 = res.tile([128, 4, DHC], F16, tag="wvt", name="wvt")
            wqT4 = res.tile([128, 4, DHC], F16, tag="wq", name="wq")
            qTt4 = res.tile([128, 4, G], F16, tag="qt", name="qt")
            kvT = [kvT4[:, d, :] for d in range(4)]
            wkT = [wkT4[:, d, :] for d in range(4)]
            wvt = [wvt4[:, d, :] for d in range(4)]
            wqT = [wqT4[:, d, :] for d in range(4)]
            qTt = [qTt4[:, d, :] for d in range(4)]
            u2 = res.tile([128, 4, HPC], F16, tag="u2")
            KT = [res.tile([128, sc], F16, tag=f"KT{p}", name=f"KT{p}") for p in range(2)]
            QT = [res.tile([128, G], F16, tag=f"QT{p}", name=f"QT{p}") for p in range(2)]
            VA = res.tile([128, nch, HPC, 65], F16, tag="va")
            nc.vector.memset(VA.rearrange("p c h e -> p (c h e)")[:, 64::65], 1.0)
            CIS = res.tile([128, nch, G], F16, tag="cis")
            dqkA = res.tile([128, nch, HPC], F32, tag="dqka")
            dqkT = res.tile([128, nch, HPC], F32, tag="dqkt")
            woT2 = res.tile([128, 2, D], F16, tag="wo", name="wo")
            woT = [woT2[:, p, :] for p in range(2)]
            out16 = res.tile([128, 8, D], F16, tag="out16", name="out16")
            aoT = res.tile([128, 16, 128], F16, tag="aot")
            ones16 = res.tile([128, 1], F16, tag="ones")
            nc.vector.memset(ones16, 1.0)
            zrow = res.tile([1, 128], F16, tag="zrow")
            nc.vector.memset(zrow, 0.0)
            ident = res.tile([128, 128], F16, tag="ident")
            # ---- input DMAs (SP HWDGE, batched) ----
            # Ordered so PE never starves: weights first (tiny), then qT
            # (Q-proj can start ~3.7us), kv in chunk-aligned quarters
            # (V/K-proj chunks dovetail with their arrival), cis staged
            # between/after kv, woT last (needed only at the tail).
            def load4(dst, src_d, ncols):
                nc.sync.dma_start(out=dst, in_=bass.AP(
                    tensor=src_d.ap().tensor, offset=0,
                    ap=[[ncols, 128], [128 * ncols, 4], [1, ncols]]))

            def load_kv(c0, c1):
                nc.sync.dma_start(out=kvT4[:, :, c0 * 128:c1 * 128],
                                  in_=bass.AP(
                    tensor=kvT_d.ap().tensor, offset=c0 * 128,
                    ap=[[sc, 128], [128 * sc, 4], [1, (c1 - c0) * 128]]))

            def load_cis(c0, cn):
                nc.sync.dma_start(
                    out=CIS[:, c0:c0 + cn, :],
                    in_=bass.AP(tensor=cis_d.ap().tensor, offset=c0 * 128 * G,
                                ap=[[G, 128], [128 * G, cn], [1, G]]))

            load_kv(0, 4)
            load4(wvt4, wv_d, DHC)
            nc.sync.dma_start(out=u2, in_=bass.AP(
                tensor=u2_d.ap().tensor, offset=0,
                ap=[[HPC, 128], [128 * HPC, 4], [1, HPC]]))
            load4(wkT4, wkT_d, DHC)
            load4(wqT4, wqT_d, DHC)
            nc.sync.dma_start(out=ident, in_=ident_d.ap())
            load_kv(4, 8)
            load4(qTt4, qT_d, G)
            load_cis(0, 4)
            load_kv(8, 12)
            load_cis(4, 4)
            load_kv(12, nch)
            load_cis(8, 4)
            load_cis(12, nch - 12)
            nc.sync.dma_start(out=woT2, in_=bass.AP(
                tensor=woT_d.ap().tensor, offset=0,
                ap=[[D, 128], [128 * D, 2], [1, D]]))

            # PE pstate warm-up: keep the tensor engine busy until the first
            # V-proj inputs land (~5.4us) so the ramp-to-2.4GHz is done
            warm = accp.tile([128, 8, 128], F32, tag="accs", name="warm")
            for _ in range(65):
                nc.tensor.matmul(warm[:, 0, 0:64], zrow, zrow[0:1, 0:64],
                                 start=True, stop=True, skip_group_check=True)

            # ---- emission plan ----
            # Phase A (projections) and the FIRST head's attention units are
            # interleaved so the element-wise engines start ~15us in instead
            # of waiting for all of phase A; kv/cis DMA staging matches.
            accs = accp.tile([128, 8, 128], F32, tag="accs")
            ao2b = res.tile([128, 2, 8, 128], F16, tag="ao2", name="ao2")

            def qproj():
                # Q: per head-pair, [128 douts, 1024 genes], pre-scaled by AS
                for p in range(2):
                    for half in range(2):
                        gs = slice(half * 512, (half + 1) * 512)
                        qs = bigp.tile([128, 512], F32, tag="ss",
                                       name=f"qps{p}{half}")
                        for d in range(4):
                            nc.tensor.matmul(
                                qs, wqT[d][:, p * 128:(p + 1) * 128],
                                qTt[d][:, gs], start=(d == 0), stop=(d == 3))
                        if p == 0:
                            nc.scalar.activation(QT[p][:, gs], qs, AF.Copy,
                                                 bias=0.0, scale=AS)
                        else:
                            nc.vector.tensor_scalar(out=QT[p][:, gs], in0=qs,
                                                    scalar1=AS, scalar2=None,
                                                    op0=ALU.mult)

            def a_block(c_lo, c_hi):
                # V + dqK per 128-chunk (one combined PSUM tile); K-proj at
                # 512 boundaries. Copies alternate ACT/DVE; dqk on GpSimd.
                for c in range(c_lo, c_hi):
                    csl = slice(c * 128, (c + 1) * 128)
                    vd = bigp.tile([128, DHC + HPC], F32, tag="ss",
                                   name=f"vd{c}")
                    for d in range(4):
                        nc.tensor.matmul(vd[:, 0:DHC], kvT[d][:, csl], wvt[d],
                                         start=(d == 0), stop=(d == 3))
                    for d in range(4):
                        nc.tensor.matmul(vd[:, DHC:DHC + HPC], kvT[d][:, csl],
                                         u2[:, d, :],
                                         start=(d == 0), stop=(d == 3))
                    if c % 2 != 0:
                        nc.scalar.activation(
                            VA[:, c, :, 0:64],
                            vd[:, 0:DHC].rearrange("p (h e) -> p h e", e=64),
                            AF.Copy, bias=0.0, scale=1.0)
                    else:
                        nc.vector.tensor_copy(
                            VA[:, c, :, 0:64],
                            vd[:, 0:DHC].rearrange("p (h e) -> p h e", e=64))
                    nc.vector.tensor_copy(dqkA[:, c, :],
                                          vd[:, DHC:DHC + HPC])
                    nc.gpsimd.tensor_scalar(out=dqkT[:, c, :],
                                            in0=dqkA[:, c, :],
                                            scalar1=float(A_T), scalar2=B_T,
                                            op0=ALU.mult, op1=ALU.add)
                    if c % 4 == 3 or c == nch - 1:
                        j = c // 4
                        jw = min(512, sc - j * 512)
                        jsl = slice(j * 512, j * 512 + jw)
                        for p in range(2):
                            kps = bigp.tile([128, 512], F32, tag="ss",
                                            name=f"kps{j}{p}")
                            for d in range(4):
                                nc.tensor.matmul(
                                    kps[:, 0:jw],
                                    wkT[d][:, p * 128:(p + 1) * 128],
                                    kvT[d][:, jsl], start=(d == 0),
                                    stop=(d == 3))
                            if p == 0:
                                nc.scalar.activation(KT[p][:, jsl],
                                                     kps[:, 0:jw], AF.Copy,
                                                     bias=0.0, scale=1.0)
                            else:
                                nc.vector.tensor_copy(KT[p][:, jsl],
                                                      kps[:, 0:jw])

            def flush(pair, hl):
                # one batched reciprocal; normalize TS split DVE/GpSimd.
                # High priority so flush ops jump the engine FIFOs ahead of
                # the next head's queued element-wise work (the next head's
                # first accumulation WAR-waits on these reads).
                with tc.high_priority(offset=220):
                    zr8 = pb.tile([128, 8], F32, tag="zr", name=f"zr{pair}{hl}")
                    nc.vector.reciprocal(
                        zr8, accs[:, :, 64:65].rearrange("p a b -> p (a b)"))
                    for gt in range(8):
                        nc.vector.tensor_scalar(
                            out=ao2b[:, pair, gt, hl * 64:(hl + 1) * 64],
                            in0=accs[:, gt, 0:64], scalar1=zr8[:, gt:gt + 1],
                            scalar2=None, op0=ALU.mult)
                        if hl == 1 and pair == 0:
                            # mid-kernel: DMA-XBAR transpose, all on the SP
                            # queue — dma_start on scalar/vector stalls that
                            # engine's sequencer ~667ns per descriptor
                            nc.sync.dma_start_transpose(aoT[:, gt, :],
                                                        ao2b[:, 0, gt, :])
                        elif hl == 1:
                            # tail: PE transpose, output into the dead accs
                            # bank (bitcast f16) — keeps the critical path off
                            # both the HWDGE queue and the ss tile ring.
                            tp = bigp.tile([128, 128], F16, tag="ss",
                                           name=f"tp{gt}")
                            nc.tensor.transpose(tp, ao2b[:, 1, gt, :], ident)
                            if gt % 2 == 0:
                                nc.scalar.activation(aoT[:, 8 + gt, :], tp,
                                                     AF.Copy, bias=0.0,
                                                     scale=1.0)
                            else:
                                nc.vector.tensor_copy(aoT[:, 8 + gt, :], tp)

            unit_no = [0]

            def emit_front(pair, c, hl):
                """Score matmuls + exp/mask for one (pair, c, hl) unit,
                processed as two independent 512-gene halves (1 PSUM bank
                each; the attn@V consumers have subtile deps per half)."""
                idx = pair * (2 * nch) + c * 2 + hl
                h = pair * 2 + hl
                hsl = slice(hl * 64, hl * 64 + 64)
                at = pb.tile([128, G], I16, tag="at", name=f"at{idx}")
                atf = at.bitcast(F16)
                kind = sched[unit_no[0]]
                unit_no[0] += 1
                et = None
                for half in range(2):
                    gs = slice(half * 512, (half + 1) * 512)
                    ss = bigp.tile([128, 512], F32, tag="ss",
                                   name=f"ss{idx}h{half}")
                    with tc.high_priority(offset=64):
                        nc.tensor.matmul(ss,
                                         KT[pair][hsl, c * 128:(c + 1) * 128],
                                         QT[pair][hsl, gs],
                                         start=True, stop=True)
                    if kind == 'trickdve':
                        nc.vector.scalar_tensor_tensor(
                            out=at[:, gs], in0=ss, scalar=dqkT[:, c, h:h + 1],
                            in1=CIS[:, c, gs], op0=ALU.add, op1=ALU.mult)
                    else:
                        if et is None:
                            et = pb.tile([128, G], F16, tag="et",
                                         name=f"et{idx}")
                        nc.scalar.activation(et[:, gs], ss, AF.Exp,
                                             bias=dqkA[:, c, h:h + 1],
                                             scale=float(1.0 / A_T))
                        if kind == 'actdve':
                            nc.vector.tensor_tensor(out=atf[:, gs],
                                                    in0=et[:, gs],
                                                    in1=CIS[:, c, gs],
                                                    op=ALU.mult)
                if kind == 'actpool':
                    # SBUF-only masks on GpSimd, per half (subtile release)
                    for half in range(2):
                        gs = slice(half * 512, (half + 1) * 512)
                        nc.gpsimd.tensor_tensor(out=atf[:, gs],
                                                in0=et[:, gs],
                                                in1=CIS[:, c, gs],
                                                op=ALU.mult)
                return atf

            def emit_back(pair, c, hl, atf):
                """attn@V + rowsum matmuls for one unit. HW start=True
                pending-zeroes the whole 2KB PSUM bank, so accumulation
                relies on the free N=1 bank-zero matmuls in zero_accs and
                always uses start=False."""
                h = pair * 2 + hl
                for gt in range(8):
                    gsl = slice(gt * 128, (gt + 1) * 128)
                    nc.tensor.matmul(accs[:, gt, 0:65], atf[:, gsl],
                                     VA[:, c, h, :],
                                     start=False, stop=(c == nch - 1),
                                     skip_group_check=True)

            pend = [None]

            def zero_accs():
                for gt0 in (0, 4):
                    nc.tensor.matmul(accs[:, gt0, 0:1], zrow, zrow[0:1, 0:1],
                                     start=True, stop=True,
                                     skip_group_check=True)

            def b_units(pair, hl, c_lo, c_hi):
                # unit i's attn@V emitted after unit i+1's scores+exp
                # (software pipelining via pend)
                for c in range(c_lo, c_hi):
                    atf = emit_front(pair, c, hl)
                    if debug and pair * (2 * nch) + c * 2 + hl in (0, 1):
                        di = pair * (2 * nch) + c * 2 + hl
                        nc.sync.dma_start(out=dbg[f'dat{di}'].ap(),
                                          in_=atf.bitcast(I16))
                    if pend[0] is not None:
                        emit_back(*pend[0])
                    pend[0] = (pair, c, hl, atf)

            def b_finish(pair, hl):
                emit_back(*pend[0])
                pend[0] = None
                flush(pair, hl)

            # interleaved A/B prologue: first head starts as soon as
            # Q/K-proj for its chunks exist
            a_block(0, 4)
            a_block(4, 8)
            qproj()
            zero_accs()
            b_units(0, 0, 0, 4)
            a_block(8, 12)
            b_units(0, 0, 4, 8)
            a_block(12, nch)
            b_units(0, 0, 8, nch)
            b_finish(0, 0)
            for pair, hl in ((0, 1), (1, 0), (1, 1)):
                zero_accs()
                b_units(pair, hl, 0, nch)
                b_finish(pair, hl)

            # ---- phase C: output projection (partial over this core's heads)
            # Output DMA batched 2 gene-tiles per descriptor.
            for gt in range(8):
                fo = bigp.tile([128, D], F32, tag="ss", name=f"fo{gt}")
                for pair in range(2):
                    nc.tensor.matmul(fo, aoT[:, pair * 8 + gt, :], woT[pair],
                                     start=(pair == 0), stop=(pair == 1))
                if gt % 2 == 0:
                    nc.scalar.activation(out16[:, gt, :], fo, AF.Copy,
                                         bias=0.0, scale=1.0)
                else:
                    nc.vector.tensor_copy(out16[:, gt, :], fo)
                if gt % 2 == 1:
                    g0 = gt - 1
                    nc.sync.dma_start(
                        out=bass.AP(tensor=out_d.ap().tensor,
                                    offset=g0 * 128 * D,
                                    ap=[[D, 128], [128 * D, 2], [1, D]]),
                        in_=out16[:, g0:g0 + 2, :])
            if debug:
                for p in range(2):
                    nc.sync.dma_start(out=dbg['dKT'][p], in_=KT[p])
                    nc.sync.dma_start(out=dbg['dQT'][p], in_=QT[p])
                nc.sync.dma_start(out=dbg['dVA'].ap(),
                                  in_=VA.rearrange("p a b e -> p (a b e)"))
                nc.sync.dma_start(out=dbg['dqa'].ap(),
                                  in_=dqkA.rearrange("p a b -> p (a b)"))
                nc.sync.dma_start(out=dbg['dqt'].ap(),
                                  in_=dqkT.rearrange("p a b -> p (a b)"))
                nc.sync.dma_start(out=dbg['dao'].ap(),
                                  in_=ao2.rearrange("p a b -> p (a b)"))
                nc.sync.dma_start(out=dbg['daoT'].ap(),
                                  in_=aoT.rearrange("p a b -> p (a b)"))

    _split_sync_waits(nc)
    nc.finalize()
    return nc


# ---------------------------------------------------------------------------
# Host-side sharding / unsharding
# ---------------------------------------------------------------------------
def make_in_maps(queries, keys_values, dq, dk, mask, cis_mask,
                 wq_w, wq_b, wk_w, wk_b, wv_w, wv_b, wo_w, wo_b, ln_g, ln_b):
    f32 = np.float32
    valid = [np.nonzero(np.asarray(mask[b]) == 1)[0] for b in range(B)]
    maxv = max(len(v) for v in valid)
    nch = max(MIN_NCH, (maxv + 127) // 128)
    sc = nch * 128

    in_maps = []
    for core in range(N_CORES):
        b, hh = core // 2, core % 2
        hsl = slice(hh * DHC, (hh + 1) * DHC)
        vb = valid[b]
        nv = len(vb)
        kvc = np.zeros((sc, D), f32)
        kvc[:nv] = np.asarray(keys_values[b], f32)[vb]
        cisk = np.zeros((sc, G), np.float16)
        cisk[:nv] = np.asarray(cis_mask)[:, vb].T.astype(np.float16)
        bvec = (np.asarray(wq_b, f32) + np.asarray(dq[b, 0], f32))[hsl]
        wk_c = np.asarray(wk_w, f32)[hsl, :]          # [256, 512]
        u2 = (np.einsum('hd,hde->eh', bvec.reshape(HPC, DK),
                        wk_c.reshape(HPC, DK, D))
              * SCALE).astype(np.float16)
        in_maps.append(dict(
            kvT=np.ascontiguousarray(kvc.T).astype(np.float16),
            qT=np.ascontiguousarray(np.asarray(queries[b], f32).T).astype(np.float16),
            wqT=np.ascontiguousarray(np.asarray(wq_w, f32)[hsl, :].T).astype(np.float16),
            wkT=np.ascontiguousarray(wk_c.T).astype(np.float16),
            wv=np.ascontiguousarray(np.asarray(wv_w, f32)[hsl, :].T).astype(np.float16),
            u2=np.ascontiguousarray(u2),
            woT=np.ascontiguousarray(np.asarray(wo_w, f32).T[hsl, :]).astype(np.float16),
            cisk=cisk,
            ident=np.eye(128, dtype=np.float16),
        ))
    return in_maps, nch


_CACHE = {}


def _run_in_maps(in_maps, nch):
    key = f"nc{nch}"
    if key not in _CACHE:
        _CACHE[key] = build_nc(nch)
    res = run_bass_kernel_spmd(_CACHE[key], in_maps,
                               core_ids=list(range(N_CORES)))
    return [r["out"] for r in res.results]


def _child_run(in_maps, nch, q):
    try:
        q.put(("ok", _run_in_maps(in_maps, nch)))
    except Exception as e:  # noqa: BLE001
        q.put(("err", repr(e)))


def kernel(**inputs):
    in_maps, nch = make_in_maps(**inputs)
    outs = None
    first_err = None
    try:
        outs = _run_in_maps(in_maps, nch)
    except Exception as e:  # noqa: BLE001
        import traceback
        first_err = traceback.format_exc()
        # A failed NEFF exec can leave this process's device client
        # unrecoverable; a fresh process (with the NEFF cached) may succeed.
        import multiprocessing as mp
        ctx = mp.get_context("spawn")
        last = None
        for _ in range(2):
            try:
                q = ctx.Queue()
                proc = ctx.Process(target=_child_run, args=(in_maps, nch, q))
                proc.start()
                status, payload = q.get()
                proc.join()
            except Exception as ce:  # noqa: BLE001
                status, payload = "err", repr(ce)
            if status == "ok":
                outs = payload
                break
            last = payload
        if outs is None:
            raise RuntimeError(
                f"kernel failed; first error:\n{first_err}\nretry: {last}")

    # host-side unshard: sum head-half partials, add biases, layernorm (f64)
    f64 = np.float64
    wo_bias = (np.asarray(inputs['wv_b'], f64) @ np.asarray(inputs['wo_w'], f64).T
               + np.asarray(inputs['wo_b'], f64))
    out = np.empty((B, G, D), np.float32)
    ln_g = np.asarray(inputs['ln_g'], f64)
    ln_b = np.asarray(inputs['ln_b'], f64)
    for b in range(B):
        acc = (np.asarray(outs[2 * b], f64) + np.asarray(outs[2 * b + 1], f64)
               + wo_bias)
        mu = acc.mean(-1, keepdims=True)
        var = acc.var(-1, keepdims=True)
        out[b] = ((acc - mu) / np.sqrt(var + 1e-5) * ln_g + ln_b).astype(np.float32)
    return out



# revision 70
# speedup vs baseline: 1.0030x; 1.0030x over previous
"""CisAttentionLayer Trainium2 kernel — 8-core SPMD via bass/Tile.

Sharding: core = (batch b, head-half hh). Each core computes 4 heads x all
1024 genes for one batch. The final output-projection partials from the two
cores of a batch are summed on the host during unshard, where the (cheap,
output-sized) layernorm also runs in f64.

Key optimizations over the dense baseline:
  - SNP compaction: the padding mask kills ~50% of the 4096 SNPs for every
    gene/head of a batch, so the host gathers only valid SNPs (padded to
    NCH*128). All attention work (projections, scores, exp, attn@V) halves.
  - Head-parallel sharding halves the K/V projections instead of
    duplicating them per gene-half.
  - attn@V computed in [genes, d_k] layout (genes on PSUM partitions,
    N=64 free) — half the PE cycles of the [d_k, genes] layout.
  - Row sums via an appended ones-column on V (65-wide attn@V matmuls).
  - Scores processed as two 512-gene halves per (pair,c,hl) unit: each
    half is one PSUM bank, giving a 6-deep ss ring (vs 3 full-width) so
    three element-wise engines stay fed; attn@V consumes per-half via
    subtile deps.
  - exp split across engines by a static balanced schedule (GPSIMD
    cannot read PSUM, so only ACT/DVE touch scores): ~42/68 units use
    ACT native exp with the cis multiply on DVE (2x-mode fp16 TT,
    per-half) or GpSimd (SBUF-only TT); ~26 units use a fused DVE
    bit-trick exp ((score*A*SCALE + A*SCALE*dqK + B) as int16, bitcast
    to fp16 ~= exp((score+dqK)*SCALE), cis multiply riding in the same
    scalar_tensor_tensor). One fp16 {0,1} mask tile serves all paths
    (and kills the compaction padding).
  - attn@V accumulators for 8 gene-tiles share PSUM banks; the bank is
    pending-zeroed with a free N=1 start=True matmul (HW start=True
    zeroes the whole 2KB bank!) and all accumulating matmuls use
    start=False.
  - Phase A (projections) and the first head's attention units are
    emitted interleaved, with kv/cis DMAs staged in chunk-aligned pieces
    so PE and the element-wise engines start as early as possible.
  - Softmax normalize: one batched reciprocal + one gt-broadcast
    (stride-0) tensor_tensor per head on DVE.
  - The dq environment shift is algebraically reduced to a per-SNP bias
    dqK[h,s] = (wq_b+dq)_h . K_h[s], computed on-device from fp16 kv with
    tiny N=4 matmuls (u_h = (wq_b+dq)_h @ wk_h precomputed on host).
  - Output transpose ([g,dk] -> [dk,g]): pair 0 via DMA-XBAR transposes
    on the SP queue mid-kernel (dma_start on scalar/vector stalls that
    engine's sequencer ~667ns); pair 1 (the tail) via PE transposes with
    a host-provided identity, keeping the critical path off the serial
    HWDGE descriptor queue. Output DMA batched 2 gene-tiles/descriptor.
"""
import numpy as np
import concourse.bass as bass
import concourse.tile as tile
from concourse import mybir
from concourse.bass_utils import run_bass_kernel_spmd
from concourse.vector_clock import ScopedClock

B, G, S, D, H, DK = 4, 1024, 4096, 512, 8, 64
N_CORES = 8
HPC = 4                 # heads per core
DHC = HPC * DK          # head dims per core (256)
SCALE = 1.0 / np.sqrt(DK).astype(np.float32)   # 0.125
A_T = 1024.0 / np.log(2.0)                     # fp16 bit-trick exp slope
B_T = 15360.0 - 45.0                           # bit-trick intercept (calibrated)
AS = float(A_T * SCALE)                        # QT pre-scale
KILL_I16 = 31743                               # int16; as fp16 bits = 65504.0
MIN_NCH = 17                                   # 17*128 = 2176 >= max valid SNPs

F32 = mybir.dt.float32
F16 = mybir.dt.float16
I16 = mybir.dt.int16
AF = mybir.ActivationFunctionType
ALU = mybir.AluOpType


# ---------------------------------------------------------------------------
# Tile compat: this container's walrus rejects >1 sync wait per instruction.
# ---------------------------------------------------------------------------
def _split_sync_waits(nc):
    for f in nc.m.functions:
        for bb in f.blocks:
            idx = 0
            while idx < len(bb.instructions):
                inst = bb.instructions[idx]
                si = inst.sync_info
                if si is not None and len(si.on_wait) > 1:
                    waits = list(si.on_wait)
                    for w in waits[:-1]:
                        nop = mybir.InstNoOp(
                            name=nc.get_next_instruction_name(),
                            sync_info=mybir.SyncInfo(on_wait=[w], on_update=[]),
                            bass_nofuse=True,
                            engine=inst.engine,
                        )
                        nc.register_instruction(nop)
                        bb.instructions.insert(idx, nop)
                        idx += 1
                    inst.sync_info = mybir.SyncInfo(
                        on_wait=[waits[-1]], on_update=list(si.on_update)
                    )
                idx += 1


class _SafeTileContext(tile.TileContext):
    def _drain_and_barrier(self, tick_clock, wait_clock):
        drain_inst = self.nc.sync.drain()
        wait_clock.add_sem_waits(
            drain_inst.ins, ScopedClock({None: tick_clock.global_clock})
        )
        si = drain_inst.ins.sync_info
        if si is not None and len(si.on_wait) > 1:
            waits = list(si.on_wait)
            drain_inst.ins.sync_info = mybir.SyncInfo(
                on_wait=[waits[0]], on_update=list(si.on_update)
            )
            for w in waits[1:]:
                extra = self.nc.sync.drain()
                extra.ins.sync_info = mybir.SyncInfo(on_wait=[w], on_update=[])
        self.nc.all_engine_barrier()
        assert self.sems is not None
        popped = self.nc._tile_sem_poison_stack.pop()
        assert popped is self._sem_poison
        self.nc.clear_and_free_semaphores(list(self.sems.allocated().values()))
        self.nc.all_engine_barrier()


# ---------------------------------------------------------------------------
# Engine assignment for the (pair,c,hl) units (each = two 512-gene halves).
#
# GPSIMD cannot read PSUM, so the score tensor is consumed only by ACT
# (exp) or DVE (fused bit-trick STT); GpSimd contributes the SBUF-only
# cis-mask multiply for ACT-path units. Types, spread evenly (Bresenham):
#   'actdve'   ACT exp x2 halves + DVE fp16 2x-mode half-masks
#   'actpool'  ACT exp x2 halves + one GpSimd full-width mask
#   'trickdve' DVE fused bit-trick STT x2 halves
# Counts solve ACT ~= DVE(+flush) ~= Pool busy time in phase B.
# ---------------------------------------------------------------------------
def _assignments(nch):
    n = 2 * nch * 2
    counts = {'actdve': 26, 'actpool': 16, 'trickdve': 26}
    if n != 68:  # rescale for other nch
        tot = sum(counts.values())
        counts = {k: max(1, round(v * n / tot)) for k, v in counts.items()}
        counts['actdve'] += n - sum(counts.values())
    sched = []
    acc = {k: 0.0 for k in counts}
    for _ in range(n):
        # pick the type furthest behind its target rate
        k = max(counts, key=lambda t: counts[t] / n * (len(sched) + 1) - acc[t])
        sched.append(k)
        acc[k] += 1
    return sched


# ---------------------------------------------------------------------------
# Kernel build
# ---------------------------------------------------------------------------
def build_nc(nch=MIN_NCH, debug=False):
    sc = nch * 128          # padded compacted SNP count
    nj = (sc + 511) // 512  # 512-wide K-proj column chunks
    sched = _assignments(nch)
    # drain: DVE also runs the last flush, so finish on ACT/Pool paths
    sched[-3:] = ['actdve', 'actpool', 'actdve']

    nc = bass.Bass()
    kvT_d = nc.dram_tensor("kvT", [D, sc], F16, kind="ExternalInput")
    qT_d = nc.dram_tensor("qT", [D, G], F16, kind="ExternalInput")
    wqT_d = nc.dram_tensor("wqT", [D, DHC], F16, kind="ExternalInput")
    wkT_d = nc.dram_tensor("wkT", [D, DHC], F16, kind="ExternalInput")
    wv_d = nc.dram_tensor("wv", [D, DHC], F16, kind="ExternalInput")
    u2_d = nc.dram_tensor("u2", [D, HPC], F16, kind="ExternalInput")
    woT_d = nc.dram_tensor("woT", [DHC, D], F16, kind="ExternalInput")
    cis_d = nc.dram_tensor("cisk", [sc, G], F16, kind="ExternalInput")
    ident_d = nc.dram_tensor("ident", [128, 128], F16, kind="ExternalInput")
    out_d = nc.dram_tensor("out", [G, D], F16, kind="ExternalOutput")
    if debug:
        dbg = {
            'dKT': nc.dram_tensor("dKT", [2, 128, sc], F16, kind="ExternalOutput"),
            'dQT': nc.dram_tensor("dQT", [2, 128, G], F16, kind="ExternalOutput"),
            'dVA': nc.dram_tensor("dVA", [128, nch * HPC * 65], F16, kind="ExternalOutput"),
            'dqa': nc.dram_tensor("dqa", [128, nch * HPC], F32, kind="ExternalOutput"),
            'dqt': nc.dram_tensor("dqt", [128, nch * HPC], F32, kind="ExternalOutput"),
            'dat0': nc.dram_tensor("dat0", [128, G], I16, kind="ExternalOutput"),
            'dat1': nc.dram_tensor("dat1", [128, G], I16, kind="ExternalOutput"),
            'dao': nc.dram_tensor("dao", [128, 8 * 128], F16, kind="ExternalOutput"),
            'daoT': nc.dram_tensor("daoT", [128, 16 * 128], F16, kind="ExternalOutput"),
        }

    with _SafeTileContext(nc) as tc:
        with tc.tile_pool(name="res", bufs=1) as res, \
             tc.tile_pool(name="pb", bufs=11) as pb, \
             tc.tile_pool(name="ao", bufs=2) as aop, \
             tc.tile_pool(name="big", bufs=6, space="PSUM") as bigp, \
             tc.tile_pool(name="acc", bufs=1, space="PSUM") as accp:

            # ---- resident SBUF tensors ----
            kvT4 = res.tile([128, 4, sc], F16, tag="kv", name="kv")
            wkT4 = res.tile([128, 4, DHC], F16, tag="wk", name="wk")
            wvt4 = res.tile([128, 4, DHC], F16, tag="wvt", name="wvt")
            wqT4 = res.tile([128, 4, DHC], F16, tag="wq", name="wq")
            qTt4 = res.tile([128, 4, G], F16, tag="qt", name="qt")
            kvT = [kvT4[:, d, :] for d in range(4)]
            wkT = [wkT4[:, d, :] for d in range(4)]
            wvt = [wvt4[:, d, :] for d in range(4)]
            wqT = [wqT4[:, d, :] for d in range(4)]
            qTt = [qTt4[:, d, :] for d in range(4)]
            u2 = res.tile([128, 4, HPC], F16, tag="u2")
            KT = [res.tile([128, sc], F16, tag=f"KT{p}", name=f"KT{p}") for p in range(2)]
            QT = [res.tile([128, G], F16, tag=f"QT{p}", name=f"QT{p}") for p in range(2)]
            VA = res.tile([128, nch, HPC, 65], F16, tag="va")
            nc.vector.memset(VA.rearrange("p c h e -> p (c h e)")[:, 64::65], 1.0)
            CIS = res.tile([128, nch, G], F16, tag="cis")
            dqkA = res.tile([128, nch, HPC], F32, tag="dqka")
            dqkT = res.tile([128, nch, HPC], F32, tag="dqkt")
            woT2 = res.tile([128, 2, D], F16, tag="wo", name="wo")
            woT = [woT2[:, p, :] for p in range(2)]
            out16 = res.tile([128, 8, D], F16, tag="out16", name="out16")
            aoT = res.tile([128, 16, 128], F16, tag="aot")
            ones16 = res.tile([128, 1], F16, tag="ones")
            nc.vector.memset(ones16, 1.0)
            zrow = res.tile([1, 128], F16, tag="zrow")
            nc.vector.memset(zrow, 0.0)
            ident = res.tile([128, 128], F16, tag="ident")
            # ---- input DMAs (SP HWDGE, batched) ----
            # Ordered so PE never starves: weights first (tiny), then qT
            # (Q-proj can start ~3.7us), kv in chunk-aligned quarters
            # (V/K-proj chunks dovetail with their arrival), cis staged
            # between/after kv, woT last (needed only at the tail).
            def load4(dst, src_d, ncols):
                nc.sync.dma_start(out=dst, in_=bass.AP(
                    tensor=src_d.ap().tensor, offset=0,
                    ap=[[ncols, 128], [128 * ncols, 4], [1, ncols]]))

            def load_kv(c0, c1):
                nc.sync.dma_start(out=kvT4[:, :, c0 * 128:c1 * 128],
                                  in_=bass.AP(
                    tensor=kvT_d.ap().tensor, offset=c0 * 128,
                    ap=[[sc, 128], [128 * sc, 4], [1, (c1 - c0) * 128]]))

            def load_cis(c0, cn):
                nc.sync.dma_start(
                    out=CIS[:, c0:c0 + cn, :],
                    in_=bass.AP(tensor=cis_d.ap().tensor, offset=c0 * 128 * G,
                                ap=[[G, 128], [128 * G, cn], [1, G]]))

            load_kv(0, 4)
            load4(wvt4, wv_d, DHC)
            nc.sync.dma_start(out=u2, in_=bass.AP(
                tensor=u2_d.ap().tensor, offset=0,
                ap=[[HPC, 128], [128 * HPC, 4], [1, HPC]]))
            load4(wkT4, wkT_d, DHC)
            load4(wqT4, wqT_d, DHC)
            nc.sync.dma_start(out=ident, in_=ident_d.ap())
            load_kv(4, 8)
            load4(qTt4, qT_d, G)
            load_cis(0, 4)
            load_kv(8, 12)
            load_cis(4, 4)
            load_kv(12, nch)
            load_cis(8, 4)
            load_cis(12, nch - 12)
            nc.sync.dma_start(out=woT2, in_=bass.AP(
                tensor=woT_d.ap().tensor, offset=0,
                ap=[[D, 128], [128 * D, 2], [1, D]]))

            # PE pstate warm-up: keep the tensor engine busy until the first
            # V-proj inputs land (~5.4us) so the ramp-to-2.4GHz is done
            warm = accp.tile([128, 8, 128], F32, tag="accs", name="warm")
            for _ in range(65):
                nc.tensor.matmul(warm[:, 0, 0:64], zrow, zrow[0:1, 0:64],
                                 start=True, stop=True, skip_group_check=True)

            # ---- emission plan ----
            # Phase A (projections) and the FIRST head's attention units are
            # interleaved so the element-wise engines start ~15us in instead
            # of waiting for all of phase A; kv/cis DMA staging matches.
            accs = accp.tile([128, 8, 128], F32, tag="accs")
            ao2b = res.tile([128, 2, 8, 128], F16, tag="ao2", name="ao2")

            def qproj():
                # Q: per head-pair, [128 douts, 1024 genes], pre-scaled by AS
                for p in range(2):
                    for half in range(2):
                        gs = slice(half * 512, (half + 1) * 512)
                        qs = bigp.tile([128, 512], F32, tag="ss",
                                       name=f"qps{p}{half}")
                        for d in range(4):
                            nc.tensor.matmul(
                                qs, wqT[d][:, p * 128:(p + 1) * 128],
                                qTt[d][:, gs], start=(d == 0), stop=(d == 3))
                        if p == 0:
                            nc.scalar.activation(QT[p][:, gs], qs, AF.Copy,
                                                 bias=0.0, scale=AS)
                        else:
                            nc.vector.tensor_scalar(out=QT[p][:, gs], in0=qs,
                                                    scalar1=AS, scalar2=None,
                                                    op0=ALU.mult)

            def a_block(c_lo, c_hi):
                # V + dqK per 128-chunk (one combined PSUM tile); K-proj at
                # 512 boundaries. Copies alternate ACT/DVE; dqk on GpSimd.
                for c in range(c_lo, c_hi):
                    csl = slice(c * 128, (c + 1) * 128)
                    vd = bigp.tile([128, DHC + HPC], F32, tag="ss",
                                   name=f"vd{c}")
                    for d in range(4):
                        nc.tensor.matmul(vd[:, 0:DHC], kvT[d][:, csl], wvt[d],
                                         start=(d == 0), stop=(d == 3))
                    for d in range(4):
                        nc.tensor.matmul(vd[:, DHC:DHC + HPC], kvT[d][:, csl],
                                         u2[:, d, :],
                                         start=(d == 0), stop=(d == 3))
                    if c % 2 != 0:
                        nc.scalar.activation(
                            VA[:, c, :, 0:64],
                            vd[:, 0:DHC].rearrange("p (h e) -> p h e", e=64),
                            AF.Copy, bias=0.0, scale=1.0)
                    else:
                        nc.vector.tensor_copy(
                            VA[:, c, :, 0:64],
                            vd[:, 0:DHC].rearrange("p (h e) -> p h e", e=64))
                    nc.vector.tensor_copy(dqkA[:, c, :],
                                          vd[:, DHC:DHC + HPC])
                    nc.gpsimd.tensor_scalar(out=dqkT[:, c, :],
                                            in0=dqkA[:, c, :],
                                            scalar1=float(A_T), scalar2=B_T,
                                            op0=ALU.mult, op1=ALU.add)
                    if c % 4 == 3 or c == nch - 1:
                        j = c // 4
                        jw = min(512, sc - j * 512)
                        jsl = slice(j * 512, j * 512 + jw)
                        for p in range(2):
                            kps = bigp.tile([128, 512], F32, tag="ss",
                                            name=f"kps{j}{p}")
                            for d in range(4):
                                nc.tensor.matmul(
                                    kps[:, 0:jw],
                                    wkT[d][:, p * 128:(p + 1) * 128],
                                    kvT[d][:, jsl], start=(d == 0),
                                    stop=(d == 3))
                            if p == 0:
                                nc.scalar.activation(KT[p][:, jsl],
                                                     kps[:, 0:jw], AF.Copy,
                                                     bias=0.0, scale=1.0)
                            else:
                                nc.vector.tensor_copy(KT[p][:, jsl],
                                                      kps[:, 0:jw])

            def flush(pair, hl):
                # one batched reciprocal; normalize TS split DVE/GpSimd.
                # High priority so flush ops jump the engine FIFOs ahead of
                # the next head's queued element-wise work (the next head's
                # first accumulation WAR-waits on these reads).
                with tc.high_priority(offset=220):
                    zr8 = pb.tile([128, 8], F32, tag="zr", name=f"zr{pair}{hl}")
                    nc.vector.reciprocal(
                        zr8, accs[:, :, 64:65].rearrange("p a b -> p (a b)"))
                    for gt in range(8):
                        nc.vector.tensor_scalar(
                            out=ao2b[:, pair, gt, hl * 64:(hl + 1) * 64],
                            in0=accs[:, gt, 0:64], scalar1=zr8[:, gt:gt + 1],
                            scalar2=None, op0=ALU.mult)
                        if hl == 1 and pair == 0:
                            # mid-kernel: DMA-XBAR transpose, all on the SP
                            # queue — dma_start on scalar/vector stalls that
                            # engine's sequencer ~667ns per descriptor
                            nc.sync.dma_start_transpose(aoT[:, gt, :],
                                                        ao2b[:, 0, gt, :])
                        elif hl == 1:
                            # tail: PE transpose, output into the dead accs
                            # bank (bitcast f16) — keeps the critical path off
                            # both the HWDGE queue and the ss tile ring.
                            tp = bigp.tile([128, 128], F16, tag="ss",
                                           name=f"tp{gt}")
                            nc.tensor.transpose(tp, ao2b[:, 1, gt, :], ident)
                            if gt % 2 == 0:
                                nc.scalar.activation(aoT[:, 8 + gt, :], tp,
                                                     AF.Copy, bias=0.0,
                                                     scale=1.0)
                            else:
                                nc.vector.tensor_copy(aoT[:, 8 + gt, :], tp)

            unit_no = [0]

            def emit_front(pair, c, hl):
                """Score matmuls + exp/mask for one (pair, c, hl) unit,
                processed as two independent 512-gene halves (1 PSUM bank
                each; the attn@V consumers have subtile deps per half)."""
                idx = pair * (2 * nch) + c * 2 + hl
                h = pair * 2 + hl
                hsl = slice(hl * 64, hl * 64 + 64)
                at = pb.tile([128, G], I16, tag="at", name=f"at{idx}")
                atf = at.bitcast(F16)
                kind = sched[unit_no[0]]
                unit_no[0] += 1
                et = None
                for half in range(2):
                    gs = slice(half * 512, (half + 1) * 512)
                    ss = bigp.tile([128, 512], F32, tag="ss",
                                   name=f"ss{idx}h{half}")
                    with tc.high_priority(offset=64):
                        nc.tensor.matmul(ss,
                                         KT[pair][hsl, c * 128:(c + 1) * 128],
                                         QT[pair][hsl, gs],
                                         start=True, stop=True)
                    if kind == 'trickdve':
                        nc.vector.scalar_tensor_tensor(
                            out=at[:, gs], in0=ss, scalar=dqkT[:, c, h:h + 1],
                            in1=CIS[:, c, gs], op0=ALU.add, op1=ALU.mult)
                    else:
                        if et is None:
                            et = pb.tile([128, G], F16, tag="et",
                                         name=f"et{idx}")
                        nc.scalar.activation(et[:, gs], ss, AF.Exp,
                                             bias=dqkA[:, c, h:h + 1],
                                             scale=float(1.0 / A_T))
                        if kind == 'actdve':
                            nc.vector.tensor_tensor(out=atf[:, gs],
                                                    in0=et[:, gs],
                                                    in1=CIS[:, c, gs],
                                                    op=ALU.mult)
                if kind == 'actpool':
                    # SBUF-only masks on GpSimd, per half (subtile release)
                    for half in range(2):
                        gs = slice(half * 512, (half + 1) * 512)
                        nc.gpsimd.tensor_tensor(out=atf[:, gs],
                                                in0=et[:, gs],
                                                in1=CIS[:, c, gs],
                                                op=ALU.mult)
                return atf

            def emit_back(pair, c, hl, atf):
                """attn@V + rowsum matmuls for one unit. HW start=True
                pending-zeroes the whole 2KB PSUM bank, so accumulation
                relies on the free N=1 bank-zero matmuls in zero_accs and
                always uses start=False."""
                h = pair * 2 + hl
                for gt in range(8):
                    gsl = slice(gt * 128, (gt + 1) * 128)
                    nc.tensor.matmul(accs[:, gt, 0:65], atf[:, gsl],
                                     VA[:, c, h, :],
                                     start=False, stop=(c == nch - 1),
                                     skip_group_check=True)

            pend = [None]

            def zero_accs():
                for gt0 in (0, 4):
                    nc.tensor.matmul(accs[:, gt0, 0:1], zrow, zrow[0:1, 0:1],
                                     start=True, stop=True,
                                     skip_group_check=True)

            def b_units(pair, hl, c_lo, c_hi):
                # unit i's attn@V emitted after unit i+1's scores+exp
                # (software pipelining via pend)
                for c in range(c_lo, c_hi):
                    atf = emit_front(pair, c, hl)
                    if debug and pair * (2 * nch) + c * 2 + hl in (0, 1):
                        di = pair * (2 * nch) + c * 2 + hl
                        nc.sync.dma_start(out=dbg[f'dat{di}'].ap(),
                                          in_=atf.bitcast(I16))
                    if pend[0] is not None:
                        emit_back(*pend[0])
                    pend[0] = (pair, c, hl, atf)

            def b_finish(pair, hl):
                emit_back(*pend[0])
                pend[0] = None
                flush(pair, hl)

            # interleaved A/B prologue: first head starts as soon as
            # Q/K-proj for its chunks exist
            a_block(0, 4)
            a_block(4, 8)
            qproj()
            zero_accs()
            b_units(0, 0, 0, 4)
            a_block(8, 12)
            b_units(0, 0, 4, 8)
            a_block(12, 14)
            b_units(0, 0, 8, 12)
            a_block(14, nch)
            b_units(0, 0, 12, nch)
            b_finish(0, 0)
            for pair, hl in ((0, 1), (1, 0), (1, 1)):
                zero_accs()
                b_units(pair, hl, 0, nch)
                b_finish(pair, hl)

            # ---- phase C: output projection (partial over this core's heads)
            # Output DMA batched 2 gene-tiles per descriptor.
            for gt in range(8):
                fo = bigp.tile([128, D], F32, tag="ss", name=f"fo{gt}")
                for pair in range(2):
                    nc.tensor.matmul(fo, aoT[:, pair * 8 + gt, :], woT[pair],
                                     start=(pair == 0), stop=(pair == 1))
                if gt % 2 == 0:
                    nc.scalar.activation(out16[:, gt, :], fo, AF.Copy,
                                         bias=0.0, scale=1.0)
                else:
                    nc.vector.tensor_copy(out16[:, gt, :], fo)
                if gt % 2 == 1:
                    g0 = gt - 1
                    nc.sync.dma_start(
                        out=bass.AP(tensor=out_d.ap().tensor,
                                    offset=g0 * 128 * D,
                                    ap=[[D, 128], [128 * D, 2], [1, D]]),
                        in_=out16[:, g0:g0 + 2, :])
            if debug:
                for p in range(2):
                    nc.sync.dma_start(out=dbg['dKT'][p], in_=KT[p])
                    nc.sync.dma_start(out=dbg['dQT'][p], in_=QT[p])
                nc.sync.dma_start(out=dbg['dVA'].ap(),
                                  in_=VA.rearrange("p a b e -> p (a b e)"))
                nc.sync.dma_start(out=dbg['dqa'].ap(),
                                  in_=dqkA.rearrange("p a b -> p (a b)"))
                nc.sync.dma_start(out=dbg['dqt'].ap(),
                                  in_=dqkT.rearrange("p a b -> p (a b)"))
                nc.sync.dma_start(out=dbg['dao'].ap(),
                                  in_=ao2.rearrange("p a b -> p (a b)"))
                nc.sync.dma_start(out=dbg['daoT'].ap(),
                                  in_=aoT.rearrange("p a b -> p (a b)"))

    _split_sync_waits(nc)
    nc.finalize()
    return nc


# ---------------------------------------------------------------------------
# Host-side sharding / unsharding
# ---------------------------------------------------------------------------
def make_in_maps(queries, keys_values, dq, dk, mask, cis_mask,
                 wq_w, wq_b, wk_w, wk_b, wv_w, wv_b, wo_w, wo_b, ln_g, ln_b):
    f32 = np.float32
    valid = [np.nonzero(np.asarray(mask[b]) == 1)[0] for b in range(B)]
    maxv = max(len(v) for v in valid)
    nch = max(MIN_NCH, (maxv + 127) // 128)
    sc = nch * 128

    in_maps = []
    for core in range(N_CORES):
        b, hh = core // 2, core % 2
        hsl = slice(hh * DHC, (hh + 1) * DHC)
        vb = valid[b]
        nv = len(vb)
        kvc = np.zeros((sc, D), f32)
        kvc[:nv] = np.asarray(keys_values[b], f32)[vb]
        cisk = np.zeros((sc, G), np.float16)
        cisk[:nv] = np.asarray(cis_mask)[:, vb].T.astype(np.float16)
        bvec = (np.asarray(wq_b, f32) + np.asarray(dq[b, 0], f32))[hsl]
        wk_c = np.asarray(wk_w, f32)[hsl, :]          # [256, 512]
        u2 = (np.einsum('hd,hde->eh', bvec.reshape(HPC, DK),
                        wk_c.reshape(HPC, DK, D))
              * SCALE).astype(np.float16)
        in_maps.append(dict(
            kvT=np.ascontiguousarray(kvc.T).astype(np.float16),
            qT=np.ascontiguousarray(np.asarray(queries[b], f32).T).astype(np.float16),
            wqT=np.ascontiguousarray(np.asarray(wq_w, f32)[hsl, :].T).astype(np.float16),
            wkT=np.ascontiguousarray(wk_c.T).astype(np.float16),
            wv=np.ascontiguousarray(np.asarray(wv_w, f32)[hsl, :].T).astype(np.float16),
            u2=np.ascontiguousarray(u2),
            woT=np.ascontiguousarray(np.asarray(wo_w, f32).T[hsl, :]).astype(np.float16),
            cisk=cisk,
            ident=np.eye(128, dtype=np.float16),
        ))
    return in_maps, nch


_CACHE = {}


def _run_in_maps(in_maps, nch):
    key = f"nc{nch}"
    if key not in _CACHE:
        _CACHE[key] = build_nc(nch)
    res = run_bass_kernel_spmd(_CACHE[key], in_maps,
                               core_ids=list(range(N_CORES)))
    return [r["out"] for r in res.results]


def _child_run(in_maps, nch, q):
    try:
        q.put(("ok", _run_in_maps(in_maps, nch)))
    except Exception as e:  # noqa: BLE001
        q.put(("err", repr(e)))


def kernel(**inputs):
    in_maps, nch = make_in_maps(**inputs)
    outs = None
    first_err = None
    try:
        outs = _run_in_maps(in_maps, nch)
    except Exception as e:  # noqa: BLE001
        import traceback
        first_err = traceback.format_exc()
        # A failed NEFF exec can leave this process's device client
        # unrecoverable; a fresh process (with the NEFF cached) may succeed.
        import multiprocessing as mp
        ctx = mp.get_context("spawn")
        last = None
        for _ in range(2):
            try:
                q = ctx.Queue()
                proc = ctx.Process(target=_child_run, args=(in_maps, nch, q))
                proc.start()
                status, payload = q.get()
                proc.join()
            except Exception as ce:  # noqa: BLE001
                status, payload = "err", repr(ce)
            if status == "ok":
                outs = payload
                break
            last = payload
        if outs is None:
            raise RuntimeError(
                f"kernel failed; first error:\n{first_err}\nretry: {last}")

    # host-side unshard: sum head-half partials, add biases, layernorm (f64)
    f64 = np.float64
    wo_bias = (np.asarray(inputs['wv_b'], f64) @ np.asarray(inputs['wo_w'], f64).T
               + np.asarray(inputs['wo_b'], f64))
    out = np.empty((B, G, D), np.float32)
    ln_g = np.asarray(inputs['ln_g'], f64)
    ln_b = np.asarray(inputs['ln_b'], f64)
    for b in range(B):
        acc = (np.asarray(outs[2 * b], f64) + np.asarray(outs[2 * b + 1], f64)
               + wo_bias)
        mu = acc.mean(-1, keepdims=True)
        var = acc.var(-1, keepdims=True)
        out[b] = ((acc - mu) / np.sqrt(var + 1e-5) * ln_g + ln_b).astype(np.float32)
    return out

